# revision 1
# baseline (speedup 1.0000x reference)
"""MetaPathAggregator kernel for Trainium2 (8 NeuronCores, data-parallel).

Math: the reference module is linear in the four gathered feature rows:

    dis  = 0.125*(mi+g1)@Wdd^T + 0.25*g2 + 0.5*dr
    drug = 0.125*(dr+g2)@Wdg^T + 0.25*g1 + 0.5*mi
    out  = [drug @ Wdrug^T | dis @ Wdis^T]
         = mi@M_mi + g1@M_g1 + g2@M_g2 + dr@M_dr

with per-slot 128x128 matrices

    M_mi = [0.500*C | 0.125*A]      A = Wdd^T @ Wdis^T   (128x64)
    M_g1 = [0.250*C | 0.125*A]      B = Wdg^T @ Wdrug^T  (128x64)
    M_g2 = [0.125*B | 0.250*D]      C = Wdrug^T          (128x64)
    M_dr = [0.125*B | 0.500*D]      D = Wdis^T           (128x64)

Since mp_ins indices are < 1000 (spec fill_max), only the first 1024 rows of
each feature table are live.  The kernel transforms the tables once on-device
(T_x = feat_x @ M_x, PE matmuls) and the per-token work collapses to four
row-gathers and three adds: out[t] = T_mi[i0]+T_g1[i1]+T_g2[i2]+T_dr[i3].

Device schedule per core (16384 tokens): prep (weights -> M matrices -> T
tables in DRAM scratch), then 16 chunks x (4 dma_gather of 1024 rows + 3 DVE
adds + 1 streaming store), with gene-table gathers software-pipelined behind
the mi/dr gathers.  HBM traffic/core ~46MB => memory(HBM-BW)-bound; the
TimelineSim cost model puts the schedule within ~10us of that roofline.
"""

import numpy as np

P = 128          # partitions
F = 128          # input feature dim
H = 128          # output hidden dim
HH = 64          # half hidden
R = 1024         # padded table rows (indices < 1000)
N_CORES = 8
B_PAIRS = 1024
BAG = 128
TOK = B_PAIRS * BAG // N_CORES   # 16384 tokens per core
CH = 1024                        # tokens per chunk (1024 descs per dma_gather)
NCH = TOK // CH                  # 16 chunks
CPB = CH // P                    # 8 tokens per partition per chunk

_CACHE = {}


def _build_module(do_gathers=True, do_adds=True, do_stores=True):
    import concourse.bacc as bacc
    import concourse.mybir as mybir
    import concourse.tile as tile
    from concourse.masks import make_identity
    from concourse.tile_rust import add_dep_helper

    f32 = mybir.dt.float32
    i16 = mybir.dt.int16

    nc = bacc.Bacc("TRN2", dynamic_dma_scratch_size=65536)

    feat_in = {
        "mi": nc.dram_tensor("feat_mi", [R, F], f32, kind="ExternalInput"),
        "ge": nc.dram_tensor("feat_ge", [R, F], f32, kind="ExternalInput"),
        "dr": nc.dram_tensor("feat_dr", [R, F], f32, kind="ExternalInput"),
    }
    w_dd = nc.dram_tensor("w_dd", [H, F], f32, kind="ExternalInput")
    w_dg = nc.dram_tensor("w_dg", [H, F], f32, kind="ExternalInput")
    w_drug = nc.dram_tensor("w_drug", [HH, F], f32, kind="ExternalInput")
    w_dis = nc.dram_tensor("w_dis", [HH, F], f32, kind="ExternalInput")
    idx_in = nc.dram_tensor("idx", [P, 4, NCH, CH // 16], i16, kind="ExternalInput")
    out = nc.dram_tensor("out", [TOK, H], f32, kind="ExternalOutput")

    with tile.TileContext(nc) as tc:
        with (
            tc.tile_pool(name="const", bufs=1) as cpool,
            tc.tile_pool(name="prep", bufs=2) as ppool,
            tc.tile_pool(name="psum", bufs=2, space="PSUM") as pspool,
            tc.tile_pool(name="tdram", bufs=1, space="DRAM") as dpool,
            tc.tile_pool(name="gather", bufs=4) as gpool,
        ):
            ident = cpool.tile([P, P], f32)
            make_identity(nc, ident[:])

            idx_t = cpool.tile([P, 4, NCH, CH // 16], i16)
            nc.sync.dma_start(idx_t[:], idx_in[:, :, :, :])

            # ---- load weights
            wdd_t = cpool.tile([H, F], f32, tag="wdd")
            nc.sync.dma_start(wdd_t[:], w_dd[:, :])
            wdg_t = cpool.tile([H, F], f32, tag="wdg")
            nc.sync.dma_start(wdg_t[:], w_dg[:, :])
            wdrug_t = cpool.tile([HH, F], f32, tag="wdrug")
            nc.sync.dma_start(wdrug_t[:], w_drug[:, :])
            wdis_t = cpool.tile([HH, F], f32, tag="wdis")
            nc.sync.dma_start(wdis_t[:], w_dis[:, :])

            # ---- C = Wdrug^T, D = Wdis^T  (PE transpose via identity)
            c_ps = pspool.tile([F, HH], f32, tag="tps")
            nc.tensor.transpose(out=c_ps[:], in_=wdrug_t[:], identity=ident[:HH, :HH])
            c_s = cpool.tile([F, HH], f32, tag="c_s")
            nc.vector.tensor_copy(out=c_s[:], in_=c_ps[:])

            d_ps = pspool.tile([F, HH], f32, tag="tps")
            nc.tensor.transpose(out=d_ps[:], in_=wdis_t[:], identity=ident[:HH, :HH])
            d_s = cpool.tile([F, HH], f32, tag="d_s")
            nc.vector.tensor_copy(out=d_s[:], in_=d_ps[:])

            # ---- A = Wdd^T @ Wdis^T, B = Wdg^T @ Wdrug^T
            a_ps = pspool.tile([F, HH], f32, tag="abps")
            nc.tensor.matmul(out=a_ps[:], lhsT=wdd_t[:], rhs=d_s[:], start=True, stop=True)
            b_ps = pspool.tile([F, HH], f32, tag="abps")
            nc.tensor.matmul(out=b_ps[:], lhsT=wdg_t[:], rhs=c_s[:], start=True, stop=True)

            # ---- assemble M matrices [F, H] in SBUF
            m = {k: cpool.tile([F, H], f32, tag=f"m_{k}", name=f"m_{k}") for k in range(4)}
            # slot 0 = mi, 1 = g1, 2 = g2, 3 = dr
            nc.vector.tensor_scalar_mul(m[0][:, :HH], c_s[:], 0.5)
            nc.vector.tensor_scalar_mul(m[0][:, HH:], a_ps[:], 0.125)
            nc.vector.tensor_scalar_mul(m[1][:, :HH], c_s[:], 0.25)
            nc.vector.tensor_scalar_mul(m[1][:, HH:], a_ps[:], 0.125)
            nc.vector.tensor_scalar_mul(m[2][:, :HH], b_ps[:], 0.125)
            nc.vector.tensor_scalar_mul(m[2][:, HH:], d_s[:], 0.25)
            nc.vector.tensor_scalar_mul(m[3][:, :HH], b_ps[:], 0.125)
            nc.vector.tensor_scalar_mul(m[3][:, HH:], d_s[:], 0.5)

            # ---- transform tables: T_k = feat @ M_k  -> DRAM scratch
            NT = R // P  # 8 row-tiles per table
            t_dram = [dpool.tile([R, F], f32, tag=f"t{k}", name=f"t_dram{k}") for k in range(4)]
            t_store = [None] * 4  # store instruction per table (for gather deps)

            feat_tiles = {}
            for name, hbm in feat_in.items():
                ft = cpool.tile([P, NT, F], f32, tag=f"feat_{name}", name=f"feat_tile_{name}")
                nc.sync.dma_start(
                    ft[:], hbm[:, :].rearrange("(r p) f -> p r f", p=P)
                )
                feat_tiles[name] = ft

            # per feature table: transpose row-tiles, then transform every slot
            # that uses it (gene feeds both g1 and g2) and store to DRAM.
            # Ordered per table so early tables' gathers can start during prep.
            feat_slots = {"mi": [0], "ge": [1, 2], "dr": [3]}
            staged = {k: ppool.tile([P, NT, F], f32, tag=f"tstage{k}", name=f"tstage{k}")
                      for k in range(4)}
            for name in ("mi", "dr", "ge"):
                for r in range(NT):
                    tp = pspool.tile([P, P], f32, tag="ftps")
                    nc.tensor.transpose(
                        out=tp[:], in_=feat_tiles[name][:, r, :], identity=ident[:]
                    )
                    fts = ppool.tile([P, P], f32, tag="ftT", name=f"ftT_{name}_{r}",
                                     bufs=3)
                    # alternate PSUM->SBUF copies between DVE and ACT
                    if r % 2 == 0:
                        nc.vector.tensor_copy(out=fts[:], in_=tp[:])
                    else:
                        nc.scalar.activation(
                            out=fts[:], in_=tp[:],
                            func=mybir.ActivationFunctionType.Copy,
                        )
                    for k in feat_slots[name]:
                        mm = pspool.tile([P, H], f32, tag="mmps")
                        nc.tensor.matmul(
                            out=mm[:], lhsT=fts[:], rhs=m[k][:],
                            start=True, stop=True,
                        )
                        if k % 2 == 0:
                            nc.vector.tensor_copy(out=staged[k][:, r, :], in_=mm[:])
                        else:
                            nc.scalar.activation(
                                out=staged[k][:, r, :], in_=mm[:],
                                func=mybir.ActivationFunctionType.Copy,
                            )
                for k in feat_slots[name]:
                    t_store[k] = nc.sync.dma_start(
                        t_dram[k][:, :].rearrange("(r p) f -> p r f", p=P),
                        staged[k][:],
                    )

            # ---- main loop: gather + add + store
            # The Pool engine runs gathers in emission order.  The gene table
            # (slots 1,2) finishes its transform last, so its gathers are
            # delayed by GE_DELAY chunks relative to mi/dr gathers -- the
            # Pool engine streams ready mi/dr gathers instead of stalling at
            # the head of the queue waiting for the gene T table.
            GE_DELAY = 5
            gtiles = {}

            def issue_gather(k, ch):
                bufs = GE_DELAY + 2 if k in (0, 3) else 5
                gt = gpool.tile([P, CPB, F], f32, tag=f"g{k}", name=f"g{k}_{ch}",
                                bufs=bufs)
                if do_gathers:
                    gi = nc.gpsimd.dma_gather(
                        gt[:], t_dram[k][:, :], idx_t[:, k, ch, :], CH, CH, F,
                    )
                    add_dep_helper(gi.ins, t_store[k].ins, reason="gather after T store")
                gtiles[(k, ch)] = gt

            for ch in range(NCH + GE_DELAY):
                if ch < NCH:
                    issue_gather(0, ch)
                    issue_gather(3, ch)
                ch2 = ch - GE_DELAY
                if 0 <= ch2 < NCH:
                    issue_gather(1, ch2)
                    issue_gather(2, ch2)
                    g = [gtiles[(k, ch2)] for k in range(4)]
                    if do_adds:
                        nc.vector.tensor_add(g[0][:], g[0][:], g[1][:])
                        nc.vector.tensor_add(g[2][:], g[2][:], g[3][:])
                        nc.vector.tensor_add(g[0][:], g[0][:], g[2][:])
                    if do_stores:
                        nc.sync.dma_start(
                            out[ch2 * CH : (ch2 + 1) * CH, :].rearrange(
                                "(p s) h -> p s h", p=P),
                            g[0][:],
                        )

    nc.compile()
    return nc


def _prep_inputs(feat_miRNA, feat_gene, feat_drug, W_drug_disease, W_disease_drug,
                 W_drug, W_dis, mp_ins):
    """Marshal full inputs into per-core in_maps (no arithmetic on values)."""
    def pad_rows(a):
        a = np.ascontiguousarray(np.asarray(a, dtype=np.float32))
        if a.shape[0] >= R:
            return np.ascontiguousarray(a[:R])
        out = np.zeros((R, a.shape[1]), dtype=np.float32)
        out[: a.shape[0]] = a
        return out

    f_mi = pad_rows(feat_miRNA)
    f_ge = pad_rows(feat_gene)
    f_dr = pad_rows(feat_drug)
    wdd = np.ascontiguousarray(np.asarray(W_drug_disease, np.float32))
    wdg = np.ascontiguousarray(np.asarray(W_disease_drug, np.float32))
    wdrug = np.ascontiguousarray(np.asarray(W_drug, np.float32))
    wdis = np.ascontiguousarray(np.asarray(W_dis, np.float32))

    mp = np.asarray(mp_ins)
    assert mp.shape == (B_PAIRS, BAG, 4), mp.shape

    # gather-slot permutation: out[p, s] holds token p*CPB+s of the chunk;
    # gather slot j = s*128+p; wrapped idx layout: j -> [j%16, j//16], x8 groups
    j = np.arange(CH)
    tok_of_j = (j % P) * CPB + (j // P)          # token within chunk for slot j

    in_maps = []
    for core in range(N_CORES):
        mp_core = mp[core * (B_PAIRS // N_CORES) : (core + 1) * (B_PAIRS // N_CORES)]
        mp_core = mp_core.reshape(TOK, 4).astype(np.int16)
        idx_arr = np.empty((P, 4, NCH, CH // 16), dtype=np.int16)
        for ch in range(NCH):
            t = ch * CH + tok_of_j                 # absolute token per slot j
            for k in range(4):
                lin = mp_core[t, k]                # idx for gather slot j
                wrapped = lin.reshape(CH // 16, 16).T   # [16, CH/16]
                idx_arr[:, k, ch, :] = np.tile(wrapped, (8, 1))
        in_maps.append(
            {
                "feat_mi": f_mi,
                "feat_ge": f_ge,
                "feat_dr": f_dr,
                "w_dd": wdd,
                "w_dg": wdg,
                "w_drug": wdrug,
                "w_dis": wdis,
                "idx": idx_arr,
            }
        )
    return in_maps


def _numpy_fallback(feat_miRNA, feat_gene, feat_drug, W_drug_disease,
                    W_disease_drug, W_drug, W_dis, mp_ins):
    mi = np.asarray(feat_miRNA, np.float32)[mp_ins[:, :, 0]]
    g1 = np.asarray(feat_gene, np.float32)[mp_ins[:, :, 1]]
    g2 = np.asarray(feat_gene, np.float32)[mp_ins[:, :, 2]]
    dr = np.asarray(feat_drug, np.float32)[mp_ins[:, :, 3]]
    wdd = np.asarray(W_drug_disease, np.float32)
    wdg = np.asarray(W_disease_drug, np.float32)
    wdrug = np.asarray(W_drug, np.float32)
    wdis = np.asarray(W_dis, np.float32)
    dis = ((((mi + g1) * 0.5) @ wdd.T + g2) * 0.5 + dr) * 0.5
    drug = ((((dr + g2) * 0.5) @ wdg.T + g1) * 0.5 + mi) * 0.5
    return np.concatenate([drug @ wdrug.T, dis @ wdis.T], axis=2)


def kernel(**inputs):
    mp = np.asarray(inputs["mp_ins"])
    if mp.max() >= R or mp.min() < 0:
        # outside the spec's index range; fall back to exact host compute
        return _numpy_fallback(**inputs)

    from concourse.bass_utils import run_bass_kernel_spmd

    if "nc" not in _CACHE:
        _CACHE["nc"] = _build_module()
    nc = _CACHE["nc"]

    in_maps = _prep_inputs(**inputs)
    res = run_bass_kernel_spmd(nc, in_maps, core_ids=list(range(N_CORES)))
    outs = [r["out"] for r in res.results]
    return np.concatenate(outs, axis=0).reshape(B_PAIRS, BAG, H)


if __name__ == "__main__":
    import reference

    inputs = {k: np.asarray(v) for k, v in reference.setup_inputs().items()}
    expected = np.asarray(reference.reference(**inputs))
    actual = kernel(**inputs)
    err = np.abs(actual - expected).max() / (np.abs(expected).max() + 1e-9)
    print("max abs err (scaled):", err)
    rel = np.linalg.norm(actual - expected) / np.linalg.norm(expected)
    print("Relative error:", rel)



# revision 2
# speedup vs baseline: 2.0783x; 2.0783x over previous
"""MetaPathAggregator kernel V2 — dual-path gather (Pool ap_gather + DMA gather).

Math (linear collapse): out[t] = T0[i0]+T1[i1]+T2[i2]+T3[i3] with
T_k = feat_k @ M_k, M_k built from the weight matrices.

Per core (TOK=16384 tokens):
- Pool path (A_TOK tokens): tables stored feature-major as PACKED bf16 pairs in
  f32 channels ([128 ch, 1024 rows] f32; ch c of half h = feats (2c,2c+1) of
  slot). One gpsimd.ap_gather per slot-PAIR (each 16-partition core group uses
  its own index stream): 2 instructions / chunk-slot-pair.
  Reduction: S1 = gA + gB (DVE bf16 add), then PE matmul with stacked identity
  (psum[c] = S1[c] + S1[64+c]), ACT psum->bf16 copy into staging, 1 store/chunk.
- DMA path (B_TOK tokens): SBUF-source transpose-mode dma_gather from a
  combined staged table [128, 32, 128] bf16 (row i at partition i%128, rank
  i//128; slot order g1,g2,mi,dr). 2 instructions x 2 slots. DVE adds, 1 store.
- DMA instruction count kept minimal (HWDGE fixed cost ~625ns each, serialized).
- Outputs bf16 feature-major; host transposes + upcasts (layout only).
"""

import numpy as np
import ml_dtypes

P = 128
F = 128
H = 128
HH = 64
R = 1024                 # padded table rows (indices < 1000)
NT = R // P              # 8 row-tiles per table
N_CORES = 8
B_PAIRS = 1024
BAG = 128
TOK = B_PAIRS * BAG // N_CORES   # 16384
A_TOK = 12800                    # pool-path tokens
B_TOK = TOK - A_TOK              # dma-path tokens
CHUNKS_A = (2048, 2048, 2048, 2048, 2048, 2048, 512)   # sums to A_TOK
assert sum(CHUNKS_A) == A_TOK
CH_D = 896                       # dma-path add chunk
NCH_D = B_TOK // CH_D
RED = 512                        # psum reduce chunk (bf16 cols)

# merged idx tensor column offsets (int16 columns)
IDX_A0 = 0
IDX_B0 = A_TOK // 16
IDX_D1 = 2 * (A_TOK // 16)
IDX_D2 = IDX_D1 + 2 * B_TOK // 16
IDX_COLS = IDX_D2 + 2 * B_TOK // 16

_CACHE = {}


def _build_module(do_pool=True, do_dma=True):
    import concourse.bacc as bacc
    import concourse.mybir as mybir
    import concourse.tile as tile
    from concourse.masks import make_identity

    f32 = mybir.dt.float32
    bf16 = mybir.dt.bfloat16
    i16 = mybir.dt.int16
    Copy = mybir.ActivationFunctionType.Copy

    nc = bacc.Bacc("TRN2", dynamic_dma_scratch_size=32768)

    feats_in = {
        "mi": nc.dram_tensor("feat_mi", [R, F], f32, kind="ExternalInput"),
        "ge": nc.dram_tensor("feat_ge", [R, F], f32, kind="ExternalInput"),
        "dr": nc.dram_tensor("feat_dr", [R, F], f32, kind="ExternalInput"),
    }
    w2_in = nc.dram_tensor("w2", [2 * H, F], f32, kind="ExternalInput")    # wdd, wdg
    wh_in = nc.dram_tensor("wh", [HH, 2 * F], f32, kind="ExternalInput")   # [wdrug|wdis]
    idx_in = nc.dram_tensor("idx", [P, IDX_COLS], i16, kind="ExternalInput")
    out_p = nc.dram_tensor("out_p", [P, A_TOK], bf16, kind="ExternalOutput")
    out_d = nc.dram_tensor("out_d", [P, B_TOK], bf16, kind="ExternalOutput")

    with tile.TileContext(nc) as tc:
        with (
            tc.tile_pool(name="const", bufs=1) as cpool,
            tc.tile_pool(name="prep", bufs=3) as ppool,
            tc.tile_pool(name="main", bufs=2) as mpool,
        ):
            import contextlib
            prep_psum = contextlib.ExitStack()
            trpool = prep_psum.enter_context(
                tc.tile_pool(name="trps", bufs=2, space="PSUM"))
            stpool = prep_psum.enter_context(
                tc.tile_pool(name="stps", bufs=2, space="PSUM"))
            pkpool = prep_psum.enter_context(
                tc.tile_pool(name="pkps", bufs=1, space="PSUM"))
            # ---------------- loads (few, large) ----------------
            wh = cpool.tile([HH, 2 * F], f32, tag="wh")
            nc.sync.dma_start(wh[:], wh_in[:, :])
            w2 = cpool.tile([P, 2, F], f32, tag="w2")
            nc.sync.dma_start(w2[:], w2_in[:, :].rearrange("(g p) f -> p g f", p=P))
            featf = {}
            for name in ("mi", "ge", "dr"):
                ft = cpool.tile([P, NT, F], f32, tag=f"featf_{name}")
                nc.sync.dma_start(
                    ft[:], feats_in[name][:, :].rearrange("(n p) f -> p n f", p=P))
                featf[name] = ft
            idx = cpool.tile([P, IDX_COLS], i16, tag="idx")
            nc.sync.dma_start(idx[:], idx_in[:, :])

            wdd_t = w2[:, 0, :]
            wdg_t = w2[:, 1, :]
            wdrug_t = wh[:, 0:F]
            wdis_t = wh[:, F:2 * F]

            # ---------------- constants ----------------
            ident = cpool.tile([P, P], f32, tag="ident")
            make_identity(nc, ident[:])
            # stacked identity [128, 64] bf16: I2[k, m] = (k % 64 == m)
            i2 = cpool.tile([P, HH], bf16, tag="i2")
            nc.vector.tensor_copy(out=i2[0:HH, :], in_=ident[0:HH, 0:HH])
            nc.sync.dma_start(i2[HH:P, :], i2[0:HH, :])

            # ---------------- weight math (f32) ----------------
            cd_ps = stpool.tile([P, 4 * P], f32, tag="stps", name="cd_ps")
            nc.tensor.transpose(out=cd_ps[:, 0:HH], in_=wdrug_t, identity=ident[:HH, :HH])
            nc.tensor.transpose(out=cd_ps[:, P:P + HH], in_=wdis_t, identity=ident[:HH, :HH])
            c_s = cpool.tile([F, HH], f32, tag="c_s")
            nc.vector.tensor_copy(out=c_s[:], in_=cd_ps[:, 0:HH])
            d_s = cpool.tile([F, HH], f32, tag="d_s")
            nc.vector.tensor_copy(out=d_s[:], in_=cd_ps[:, P:P + HH])
            ab_ps = stpool.tile([P, 4 * P], f32, tag="stps", name="ab_ps")
            nc.tensor.matmul(out=ab_ps[:, 0:HH], lhsT=wdd_t, rhs=d_s[:], start=True, stop=True)
            nc.tensor.matmul(out=ab_ps[:, P:P + HH], lhsT=wdg_t, rhs=c_s[:], start=True, stop=True)
            a_s = cpool.tile([F, HH], f32, tag="a_s")
            nc.scalar.activation(out=a_s[:], in_=ab_ps[:, 0:HH], func=Copy)
            b_s = cpool.tile([F, HH], f32, tag="b_s")
            nc.scalar.activation(out=b_s[:], in_=ab_ps[:, P:P + HH], func=Copy)

            # ---------------- M matrices (bf16) ----------------
            pieces = {0: (c_s, 0.5, a_s, 0.125), 1: (c_s, 0.25, a_s, 0.125),
                      2: (b_s, 0.125, d_s, 0.25), 3: (b_s, 0.125, d_s, 0.5)}
            m_full, m_ev, m_od = {}, {}, {}
            for k in range(4):
                lo, slo, hi, shi = pieces[k]
                mk = cpool.tile([F, H], bf16, tag=f"m{k}")
                nc.scalar.activation(out=mk[:, :HH], in_=lo[:], func=Copy,
                                     scale=float(slo))
                nc.scalar.activation(out=mk[:, HH:], in_=hi[:], func=Copy,
                                     scale=float(shi))
                m_full[k] = mk
                lo3 = lo[:].rearrange("p (r two) -> p r two", two=2)
                hi3 = hi[:].rearrange("p (r two) -> p r two", two=2)
                ev = cpool.tile([F, HH], bf16, tag=f"mev{k}")
                nc.vector.tensor_scalar_mul(ev[:, 0:32], lo3[:, :, 0], slo)
                nc.vector.tensor_scalar_mul(ev[:, 32:64], hi3[:, :, 0], shi)
                od = cpool.tile([F, HH], bf16, tag=f"mod{k}")
                nc.vector.tensor_scalar_mul(od[:, 0:32], lo3[:, :, 1], slo)
                nc.vector.tensor_scalar_mul(od[:, 32:64], hi3[:, :, 1], shi)
                m_ev[k] = ev
                m_od[k] = od

            # ---------------- table transforms (4-row-tile batches) ----------
            # staged slot order (mi, g1, g2, dr) -> rank bases
            # gather1 = ranks 0:16 (mi, g1) ready mid-prep; gather2 = 16:32
            rank_base = {0: 0, 1: 8, 2: 16, 3: 24}
            dstag = cpool.tile([P, 32, P], bf16, tag="dstag")
            a_pack = cpool.tile([P, R], f32, tag="apack")
            b_pack = cpool.tile([P, R], f32, tag="bpack")
            packs = {0: (a_pack, 0), 1: (a_pack, 1), 2: (b_pack, 0), 3: (b_pack, 1)}

            slot_feat = {0: "mi", 1: "ge", 2: "ge", 3: "dr"}
            W = 4 * P  # batch width (4 row-tiles)
            fts_t = {}
            alt = [0]

            def get_fts(name, b):
                if (name, b) in fts_t:
                    return fts_t[(name, b)]
                ts0 = 4 * b
                tr = trpool.tile([P, W], f32, tag="ftps", name=f"tr_{name}_{b}")
                for i in range(4):
                    nc.tensor.transpose(
                        out=tr[:, i * P:(i + 1) * P],
                        in_=featf[name][:, ts0 + i, :], identity=ident[:])
                fts = ppool.tile([P, W], bf16, tag=f"fts_{name}{b}",
                                 name=f"fts_{name}_{b}")
                if alt[0] % 2 == 0:
                    nc.vector.tensor_copy(out=fts[:], in_=tr[:])
                else:
                    nc.scalar.activation(out=fts[:], in_=tr[:], func=Copy)
                alt[0] += 1
                fts_t[(name, b)] = fts
                return fts

            def do_packed(k, b):
                fts = get_fts(slot_feat[k], b)
                dest, half = packs[k]
                h0 = half * HH
                pe_ps = pkpool.tile([P, W], f32, tag="pkev", name=f"pe_{k}_{b}")
                po_ps = pkpool.tile([P, W], f32, tag="pkod", name=f"po_{k}_{b}")
                for i in range(4):
                    cs = slice(i * P, (i + 1) * P)
                    nc.tensor.matmul(out=pe_ps[h0:h0 + HH, cs],
                                     lhsT=m_ev[k][:], rhs=fts[:, cs],
                                     start=True, stop=True)
                    nc.tensor.matmul(out=po_ps[h0:h0 + HH, cs],
                                     lhsT=m_od[k][:], rhs=fts[:, cs],
                                     start=True, stop=True)
                dv = dest[:].bitcast(bf16).rearrange("p (r two) -> p r two", two=2)
                rs = slice(4 * b * P, (4 * b + 4) * P)
                nc.vector.tensor_copy(out=dv[h0:h0 + HH, rs, 0],
                                      in_=pe_ps[h0:h0 + HH, :])
                nc.scalar.activation(out=dv[h0:h0 + HH, rs, 1],
                                     in_=po_ps[h0:h0 + HH, :], func=Copy)

            def do_staged(k, b):
                fts = get_fts(slot_feat[k], b)
                ts0 = 4 * b
                st_ps = stpool.tile([P, W], f32, tag="stps", name=f"st_{k}_{b}")
                for i in range(4):
                    cs = slice(i * P, (i + 1) * P)
                    nc.tensor.matmul(out=st_ps[:, cs], lhsT=fts[:, cs],
                                     rhs=m_full[k][:], start=True, stop=True)
                if (k + b) % 2 == 0:
                    nc.vector.tensor_copy(
                        out=dstag[:, rank_base[k] + ts0:rank_base[k] + ts0 + 4, :],
                        in_=st_ps[:])
                else:
                    nc.scalar.activation(
                        out=dstag[:, rank_base[k] + ts0:rank_base[k] + ts0 + 4, :],
                        in_=st_ps[:], func=Copy)

            # pass 1: a_pack (slots 0, 1) -- unblocks the pool-path gathers
            for k in (0, 1):
                for b in range(2):
                    do_packed(k, b)
            # pass 2: b_pack (slots 2, 3), then staged tables in gather order
            for k in (2, 3):
                for b in range(2):
                    do_packed(k, b)
            for k in (0, 1, 2, 3):
                for b in range(2):
                    do_staged(k, b)

            # close prep PSUM pools; main loop gets a deep reduce ring
            prep_psum.close()
            rd_psum = contextlib.ExitStack()
            rdpool = rd_psum.enter_context(
                tc.tile_pool(name="rdps", bufs=6, space="PSUM"))

            from concourse.tile_rust import add_dep_helper

            # ---------------- pool-path chunks ----------------
            s1_adds = []
            ap_instrs = []

            def pool_chunk(c, off, size):
                cols = slice(IDX_A0 + off // 16, IDX_A0 + (off + size) // 16)
                colsb = slice(IDX_B0 + off // 16, IDX_B0 + (off + size) // 16)
                CM = max(CHUNKS_A)
                ga_f = mpool.tile([P, CM], f32, tag="ga", name=f"ga{c}", bufs=4)
                ga = ga_f[:, :size]
                ap_instrs.append(nc.gpsimd.ap_gather(
                    ga, a_pack[:], idx[:, cols], P, R, 1, size))
                gb_f = mpool.tile([P, CM], f32, tag="gb", name=f"gb{c}", bufs=4)
                gb = gb_f[:, :size]
                ap_instrs.append(nc.gpsimd.ap_gather(
                    gb, b_pack[:], idx[:, colsb], P, R, 1, size))
                ga_bf = ga.bitcast(bf16)
                gb_bf = gb.bitcast(bf16)
                # staging [128, size]: even RED blocks in parts 0:64, odd 64:128
                stg_f = mpool.tile([P, CM], bf16, tag="stg", name=f"stg{c}")
                stg = stg_f[:, :size]
                nred = 2 * size // RED
                for j in range(0, nred, 2):
                    # two reduce results share one [128, RED] psum (halves);
                    # PSUM accumulates ga + gb (all four slot-tables)
                    ps = rdpool.tile([P, RED], f32, tag="rd", name=f"rd{c}_{j}")
                    jj = j // 2
                    cs = slice(jj * RED, (jj + 1) * RED)
                    for h, jx in ((0, j), (1, j + 1)):
                        hs = slice(h * HH, (h + 1) * HH)
                        sl = slice(jx * RED, (jx + 1) * RED)
                        nc.tensor.matmul(out=ps[hs, :], lhsT=i2[:],
                                         rhs=ga_bf[:, sl],
                                         start=True, stop=False)
                        nc.tensor.matmul(out=ps[hs, :], lhsT=i2[:],
                                         rhs=gb_bf[:, sl],
                                         start=False, stop=True)
                    if jj % 2 == 0:
                        nc.scalar.activation(out=stg[0:HH, cs], in_=ps[0:HH, :],
                                             func=Copy)
                        nc.scalar.activation(out=stg[HH:P, cs], in_=ps[HH:P, :],
                                             func=Copy)
                    else:
                        nc.vector.tensor_copy(out=stg[0:HH, cs], in_=ps[0:HH, :])
                        nc.vector.tensor_copy(out=stg[HH:P, cs], in_=ps[HH:P, :])
                nc.sync.dma_start(out_p[:, off:off + size], stg)

            # first pool chunk emitted before the desc-gens; the desc-gens get
            # explicit deps on the first aps so the scheduler cannot place the
            # (late-ready) desc-gens at the head of the Pool stream.
            off = 0
            if do_pool:
                pool_chunk(0, 0, CHUNKS_A[0])
                off = CHUNKS_A[0]

            g_d1 = cpool.tile([P, 1, 2 * B_TOK], bf16, tag="gd1")
            g_d2 = cpool.tile([P, 1, 2 * B_TOK], bf16, tag="gd2")
            if do_dma:
                gi1 = nc.gpsimd.dma_gather(
                    g_d1[:], dstag[:, 0:16, :],
                    idx[:, IDX_D1:IDX_D1 + 2 * B_TOK // 16],
                    2 * B_TOK, 2 * B_TOK, P,
                    transpose=True, single_packet=False,
                    sbuf_tokens_per_rank=128, sbuf_free_dim_per_rank=256,
                    sbuf_free_dim_pad_per_rank=0, sbuf_byte_offset=0,
                )
                gi2 = nc.gpsimd.dma_gather(
                    g_d2[:], dstag[:, 16:32, :],
                    idx[:, IDX_D2:IDX_D2 + 2 * B_TOK // 16],
                    2 * B_TOK, 2 * B_TOK, P,
                    transpose=True, single_packet=False,
                    sbuf_tokens_per_rank=128, sbuf_free_dim_per_rank=256,
                    sbuf_free_dim_pad_per_rank=0, sbuf_byte_offset=0,
                )
                if do_pool and ap_instrs:
                    add_dep_helper(gi1.ins, ap_instrs[0].ins, reason="pool order")
                    add_dep_helper(gi2.ins, ap_instrs[1].ins, reason="pool order")

            if do_pool:
                for c in range(1, len(CHUNKS_A) - 2):
                    pool_chunk(c, off, CHUNKS_A[c])
                    off += CHUNKS_A[c]

            # ---------------- DMA-path adds + one store ----------------
            # emitted before the last pool chunks so the out_d store is not
            # queued behind the final out_p stores on SP
            if do_dma:
                sd = cpool.tile([P, B_TOK], bf16, tag="sd")
                for c in range(NCH_D):
                    sl = slice(c * CH_D, (c + 1) * CH_D)
                    sl2 = slice(B_TOK + c * CH_D, B_TOK + (c + 1) * CH_D)
                    t01 = mpool.tile([P, CH_D], bf16, tag="t01", name=f"t01_{c}")
                    i1a = nc.vector.tensor_add(t01[:], g_d1[:, 0, sl], g_d1[:, 0, sl2])
                    t23 = mpool.tile([P, CH_D], bf16, tag="t23", name=f"t23_{c}")
                    i2a = nc.vector.tensor_add(t23[:], g_d2[:, 0, sl], g_d2[:, 0, sl2])
                    nc.vector.tensor_add(sd[:, sl], t01[:], t23[:])
                    if do_pool and ap_instrs:
                        # keep DVE from head-of-line blocking behind the big
                        # gather transfers: spread these behind pool chunks
                        anchor = ap_instrs[min(2 * c + 1, len(ap_instrs) - 1)]
                        add_dep_helper(i1a.ins, anchor.ins, reason="dve order")
                        add_dep_helper(i2a.ins, anchor.ins, reason="dve order")
                    if c == NCH_D // 2 - 1:
                        nc.sync.dma_start(out_d[:, :B_TOK // 2],
                                          sd[:, :B_TOK // 2])
                nc.sync.dma_start(out_d[:, B_TOK // 2:], sd[:, B_TOK // 2:])

            if do_pool:
                for c in range(len(CHUNKS_A) - 2, len(CHUNKS_A)):
                    pool_chunk(c, off, CHUNKS_A[c])
                    off += CHUNKS_A[c]

            rd_psum.close()

    nc.compile()
    return nc


def _wrap16(v):
    """token j -> [j % 16, j // 16] layout."""
    return np.ascontiguousarray(v.reshape(-1, 16).T)


def _prep_inputs(feat_miRNA, feat_gene, feat_drug, W_drug_disease, W_disease_drug,
                 W_drug, W_dis, mp_ins):
    def pad_rows(a):
        a = np.ascontiguousarray(np.asarray(a, dtype=np.float32))
        if a.shape[0] >= R:
            return np.ascontiguousarray(a[:R])
        out = np.zeros((R, a.shape[1]), dtype=np.float32)
        out[: a.shape[0]] = a
        return out

    f_mi = pad_rows(feat_miRNA)
    f_ge = pad_rows(feat_gene)
    f_dr = pad_rows(feat_drug)
    w2 = np.concatenate([
        np.asarray(W_drug_disease, np.float32),
        np.asarray(W_disease_drug, np.float32)], axis=0)
    w2 = np.ascontiguousarray(w2)
    wh = np.concatenate([
        np.asarray(W_drug, np.float32), np.asarray(W_dis, np.float32)], axis=1)
    wh = np.ascontiguousarray(wh)

    mp = np.asarray(mp_ins)
    assert mp.shape == (B_PAIRS, BAG, 4), mp.shape

    in_maps = []
    for core in range(N_CORES):
        mp_core = mp[core * (B_PAIRS // N_CORES):(core + 1) * (B_PAIRS // N_CORES)]
        mp_core = mp_core.reshape(TOK, 4).astype(np.int16)
        i0, i1, i2, i3 = (mp_core[:, k] for k in range(4))
        idx = np.empty((P, IDX_COLS), dtype=np.int16)
        # pool path: first A_TOK tokens; A = (i0 -> cores 0-3, i1 -> cores 4-7)
        idx[0:64, IDX_A0:IDX_B0] = np.tile(_wrap16(i0[:A_TOK]), (4, 1))
        idx[64:128, IDX_A0:IDX_B0] = np.tile(_wrap16(i1[:A_TOK]), (4, 1))
        idx[0:64, IDX_B0:IDX_D1] = np.tile(_wrap16(i2[:A_TOK]), (4, 1))
        idx[64:128, IDX_B0:IDX_D1] = np.tile(_wrap16(i3[:A_TOK]), (4, 1))
        # dma path: last B_TOK tokens; staged slot order (mi, g1, g2, dr)
        d1 = np.concatenate([i0[A_TOK:], R + i1[A_TOK:]]).astype(np.int16)
        d2 = np.concatenate([i2[A_TOK:], R + i3[A_TOK:]]).astype(np.int16)
        idx[:, IDX_D1:IDX_D2] = np.tile(_wrap16(d1), (8, 1))
        idx[:, IDX_D2:IDX_COLS] = np.tile(_wrap16(d2), (8, 1))
        in_maps.append({"feat_mi": f_mi, "feat_ge": f_ge, "feat_dr": f_dr,
                        "w2": w2, "wh": wh, "idx": idx})
    return in_maps


def _assemble(results):
    outs = []
    for r in results:
        op = np.asarray(r["out_p"]).astype(np.float32)      # [128, A_TOK]
        od = np.asarray(r["out_d"]).astype(np.float32)      # [128, B_TOK]
        # per chunk: op[h*64+cf, off + jj*RED + 2*s2 + l] = feat(2cf+l) of
        # token off + jj*512 + h*256 + s2
        parts = []
        off = 0
        for L in CHUNKS_A:
            a = op[:, off:off + L].reshape(2, HH, L // RED, RED // 2, 2)
            a = a.transpose(2, 0, 3, 1, 4)                  # [jj, h, s2, cf, l]
            parts.append(a.reshape(L, H))
            off += L
        pool = np.concatenate(parts, axis=0)
        outs.append(np.concatenate([pool, od.T], axis=0))
    return np.concatenate(outs, axis=0).reshape(B_PAIRS, BAG, H)


def _numpy_fallback(feat_miRNA, feat_gene, feat_drug, W_drug_disease,
                    W_disease_drug, W_drug, W_dis, mp_ins):
    mi = np.asarray(feat_miRNA, np.float32)[mp_ins[:, :, 0]]
    g1 = np.asarray(feat_gene, np.float32)[mp_ins[:, :, 1]]
    g2 = np.asarray(feat_gene, np.float32)[mp_ins[:, :, 2]]
    dr = np.asarray(feat_drug, np.float32)[mp_ins[:, :, 3]]
    wdd = np.asarray(W_drug_disease, np.float32)
    wdg = np.asarray(W_disease_drug, np.float32)
    wdrug = np.asarray(W_drug, np.float32)
    wdis = np.asarray(W_dis, np.float32)
    dis = ((((mi + g1) * 0.5) @ wdd.T + g2) * 0.5 + dr) * 0.5
    drug = ((((dr + g2) * 0.5) @ wdg.T + g1) * 0.5 + mi) * 0.5
    return np.concatenate([drug @ wdrug.T, dis @ wdis.T], axis=2)


def kernel(**inputs):
    mp = np.asarray(inputs["mp_ins"])
    if mp.max() >= R or mp.min() < 0:
        return _numpy_fallback(**inputs)

    from concourse.bass_utils import run_bass_kernel_spmd

    if "nc" not in _CACHE:
        _CACHE["nc"] = _build_module()
    nc = _CACHE["nc"]

    in_maps = _prep_inputs(**inputs)
    res = run_bass_kernel_spmd(nc, in_maps, core_ids=list(range(N_CORES)))
    return _assemble(res.results)


if __name__ == "__main__":
    import reference

    inputs = {k: np.asarray(v) for k, v in reference.setup_inputs().items()}
    expected = np.asarray(reference.reference(**inputs))
    actual = kernel(**inputs)
    rel = np.linalg.norm(actual - expected) / np.linalg.norm(expected)
    print("Relative error:", rel)

    from concourse.timeline_sim import TimelineSim
    print("TimelineSim:", TimelineSim(_CACHE["nc"], trace=False).simulate(), "ns")
